# revision 14
# baseline (speedup 1.0000x reference)
"""Trainium2 Bass kernel for nn_CrossAttentionLayer (ragged cross-attention).

Sharding: data-parallel over the 16 ragged samples -> 2 samples per core
(8 cores). Small attention weights replicated. Host pre-shards source by
batch_offsets (each sample's rows are contiguous), pre-transposes each
core's kv slice to [D, T] and casts to bf16. Device does K/V projections,
scores, softmax (no max-subtraction; additive -1e30 mask fused into the
exp's bias operand), attn@V with fp32 PSUM accumulation, out-projection
and residual. Softmax normalization (division by the per-query sum of
exp) happens at slot finalize on-device.
"""
import sys
import os
import numpy as np

sys.path.insert(0, "/opt/trn_rl_repo")

import ml_dtypes  # noqa: E402

BF16 = ml_dtypes.bfloat16

D = 256
H = 8
HD = 32
NQ = 100
NCORES = 8
S = 2  # sample slots per core

_prog_cache = {}
TRACE_SIM = False


def _ceil_to(x, m):
    return ((x + m - 1) // m) * m


def _patch_tile_drain():
    """walrus CoreV3 CTRL codegen rejects >2 sem-waits on one Drain; the
    Tile kernel-tail drain aggregates one wait per live proc. Split the
    waits across preceding single-wait SP nops instead."""
    from concourse import mybir
    from concourse import tile as tile_mod

    if getattr(tile_mod.TileContext, "_drain_patched", False):
        return

    def _drain_and_barrier(self, tick_clock, wait_clock):
        nc = self.nc
        carrier = nc.sync.nop(nofuse=True)
        wait_clock.add_sem_waits(
            carrier.ins, tile_mod.ScopedClock({None: tick_clock.global_clock}))
        si = carrier.ins.sync_info
        waits = list(si.on_wait) if si and si.on_wait else []
        MAXW = 1
        if len(waits) > MAXW:
            si.on_wait = waits[:MAXW]
            for i in range(MAXW, len(waits), MAXW):
                nop = nc.sync.nop(nofuse=True)
                nop.ins.sync_info = mybir.SyncInfo(
                    on_wait=waits[i:i + MAXW], on_update=[])
        nc.sync.drain()
        nc.all_engine_barrier()
        popped = nc._tile_sem_poison_stack.pop()
        assert popped is self._sem_poison
        nc.clear_and_free_semaphores(list(self.sems.allocated().values()))
        nc.all_engine_barrier()

    tile_mod.TileContext._drain_and_barrier = _drain_and_barrier
    tile_mod.TileContext._drain_patched = True


def _split_bir_waits(m, maxw=1):
    """walrus CoreV2/V3 codegen rejects instructions carrying more than one
    sync-wait command. Hoist extra waits onto same-engine NoOps inserted
    immediately before the instruction (engine execution is in-order, so
    the happens-before is preserved)."""
    uid = [0]
    for fn in m.get("functions", []):
        for bb in fn.get("blocks", []):
            out = []
            for ins in bb.get("instructions", []):
                si = ins.get("sync_info")
                waits = (si or {}).get("on_wait") or []
                if len(waits) > maxw:
                    for i in range(0, len(waits) - maxw, maxw):
                        uid[0] += 1
                        out.append({
                            "debug": ins.get("debug", 0),
                            "engine": ins["engine"],
                            "ins": [],
                            "name": f"{ins['name']}-w{uid[0]}",
                            "opcode": "NoOp",
                            "outs": [],
                            "sync_info": {
                                "on_update": [],
                                "on_wait": waits[i:i + maxw],
                            },
                        })
                    si["on_wait"] = waits[len(waits) - maxw:]
                out.append(ins)
            bb["instructions"] = out
    return m


def _install_wait_split(nc):
    import orjson
    orig = nc.to_json_bytes

    def patched():
        return orjson.dumps(_split_bir_waits(orjson.loads(orig())))

    nc.to_json_bytes = patched


def _build_program(Lslot, use_bk, use_bv, use_bq, use_bo):
    """Build the SPMD Bass program for one core handling S=2 slots of
    Lslot (multiple of 256) padded kv tokens each."""
    from concourse import bass, mybir
    from concourse.tile import TileContext

    _patch_tile_drain()

    f32 = mybir.dt.float32
    bf16 = mybir.dt.bfloat16
    Exp = mybir.ActivationFunctionType.Exp

    NB = Lslot // 128          # 128-token blocks per slot
    NIT = Lslot // 256         # 256-token iterations per slot
    T = S * Lslot
    NT = S * NB

    nc = bass.Bass()

    kvT_d = nc.declare_dram_parameter("kvT", [D, T], bf16, isOutput=False)
    qTin_d = nc.declare_dram_parameter("qTin", [D, S * NQ], bf16, isOutput=False)
    qres_d = nc.declare_dram_parameter("qres", [S * NQ, D], f32, isOutput=False)
    maskb_d = nc.declare_dram_parameter("maskb", [128, NT], f32, isOutput=False)
    wkT_d = nc.declare_dram_parameter("wkT", [D, D], bf16, isOutput=False)
    wvT_d = nc.declare_dram_parameter("wvT", [D, D], bf16, isOutput=False)
    wqT_d = nc.declare_dram_parameter("wqT", [D, D], bf16, isOutput=False)
    woT_d = nc.declare_dram_parameter("woT", [D, D], bf16, isOutput=False)
    ones_d = nc.declare_dram_parameter("onesb", [128, 256], bf16, isOutput=False)
    onesf_d = nc.declare_dram_parameter("onesf", [128, 128], f32, isOutput=False)
    bk_d = nc.declare_dram_parameter("bk", [1, D], bf16, isOutput=False)
    bv_d = nc.declare_dram_parameter("bv", [1, D], bf16, isOutput=False)
    bq_d = nc.declare_dram_parameter("bq", [1, D], bf16, isOutput=False)
    bo_d = nc.declare_dram_parameter("bo", [1, D], bf16, isOutput=False)
    out_d = nc.declare_dram_parameter("out", [S * NQ, D], f32, isOutput=True)

    with TileContext(nc, trace_sim=TRACE_SIM) as tc:
        with tc.tile_pool(name="const", bufs=1) as cpool, \
             tc.tile_pool(name="proj", bufs=2, space="PSUM") as projp, \
             tc.tile_pool(name="sp", bufs=2, space="PSUM") as spp, \
             tc.tile_pool(name="cp", bufs=2, space="PSUM") as cpp, \
             tc.tile_pool(name="lp", bufs=1, space="PSUM") as lpp, \
             tc.tile_pool(name="load", bufs=3) as loadp, \
             tc.tile_pool(name="work", bufs=2) as workp, \
             tc.tile_pool(name="pbuf", bufs=3) as pbufp:

            # ---- constants / small tensors ----
            wk_sb = cpool.tile([128, 512], bf16)
            wv_sb = cpool.tile([128, 512], bf16)
            wq_sb = cpool.tile([128, 512], bf16)
            wo_sb = cpool.tile([128, 512], bf16)
            qTin_sb = cpool.tile([128, 2 * S * NQ], bf16)
            qres_sb = cpool.tile([128, S * D], f32)
            maskb_sb = cpool.tile([128, NT], f32)
            ones_sb = cpool.tile([128, 256], bf16)
            onesf_sb = cpool.tile([128, 128], f32)
            bk_sb = cpool.tile([1, D], bf16)
            bv_sb = cpool.tile([1, D], bf16)
            bq_sb = cpool.tile([1, D], bf16)
            bo_sb = cpool.tile([1, D], bf16)
            qTz = cpool.tile([128, S * H * NQ], bf16)
            out_sb = cpool.tile([128, S * D], f32)

            for w_sb, w_d in ((wk_sb, wkT_d), (wv_sb, wvT_d),
                              (wq_sb, wqT_d), (wo_sb, woT_d)):
                nc.sync.dma_start(
                    out=w_sb[:].rearrange("p (kh d) -> p kh d", kh=2),
                    in_=w_d[:].rearrange("(kh p) d -> p kh d", kh=2))
            nc.sync.dma_start(
                out=qTin_sb[:].rearrange("p (kh n) -> p kh n", kh=2),
                in_=qTin_d[:].rearrange("(kh p) n -> p kh n", kh=2))
            nc.sync.dma_start(
                out=qres_sb[0:NQ, :].rearrange("n (s d) -> n s d", s=S),
                in_=qres_d[:].rearrange("(s n) d -> n s d", s=S))
            nc.sync.dma_start(out=maskb_sb[:], in_=maskb_d[:])
            nc.sync.dma_start(out=ones_sb[:], in_=ones_d[:])
            nc.sync.dma_start(out=onesf_sb[:], in_=onesf_d[:])
            for b_sb, b_d in ((bk_sb, bk_d), (bv_sb, bv_d),
                              (bq_sb, bq_d), (bo_sb, bo_d)):
                nc.sync.dma_start(out=b_sb[:], in_=b_d[:])

            # ---- q projection (once): qT[dout, n] over dh halves ----
            qp = spp.tile([128, 512], f32, tag="sp", name="qp")
            NQS = S * NQ
            for dh in range(2):
                for kh in range(2):
                    nc.tensor.matmul(
                        out=qp[:, dh * NQS:(dh + 1) * NQS],
                        lhsT=wq_sb[:, kh * 256 + dh * 128: kh * 256 + dh * 128 + 128],
                        rhs=qTin_sb[:, kh * NQS:(kh + 1) * NQS],
                        start=(kh == 0), stop=(kh == 1 and not use_bq))
                if use_bq:
                    nc.tensor.matmul(
                        out=qp[:, dh * NQS:(dh + 1) * NQS],
                        lhsT=bq_sb[0:1, dh * 128: dh * 128 + 128],
                        rhs=ones_sb[0:1, 0:NQS],
                        start=False, stop=True)
            nc.gpsimd.memset(qTz[:], 0.0)
            for s in range(S):
                for h in range(H):
                    rp = (h % 4) * 32
                    nc.vector.tensor_copy(
                        qTz[rp:rp + 32, (s * H + h) * NQ:(s * H + h) * NQ + NQ],
                        qp[rp:rp + 32, (h // 4) * NQS + s * NQ:
                           (h // 4) * NQS + s * NQ + NQ])

            # ---- main loop over slots and 256-token iterations ----
            for s in range(S):
                cp = [cpp.tile([128, 512], f32, tag="cp", name=f"cp_s{s}_t{t}")
                      for t in range(2)]
                lp = lpp.tile([128, 1024], f32, tag="lp", name=f"lp_s{s}")
                for it in range(NIT):
                    c0 = s * Lslot + it * 256
                    kv_sb = loadp.tile([128, 512], bf16, tag="kv")
                    nc.sync.dma_start(
                        out=kv_sb[:].rearrange("p (kh m) -> p kh m", kh=2),
                        in_=kvT_d[:, c0:c0 + 256].rearrange(
                            "(kh p) m -> p kh m", kh=2))

                    # K projection -> kT (dh-major: [dh*256 + m(2blk)])
                    kp = projp.tile([128, 512], f32, tag="proj")
                    for dh in range(2):
                        for kh in range(2):
                            nc.tensor.matmul(
                                out=kp[:, dh * 256:(dh + 1) * 256],
                                lhsT=wk_sb[:, kh * 256 + dh * 128:
                                           kh * 256 + dh * 128 + 128],
                                rhs=kv_sb[:, kh * 256:(kh + 1) * 256],
                                start=(kh == 0), stop=(kh == 1 and not use_bk))
                        if use_bk:
                            nc.tensor.matmul(
                                out=kp[:, dh * 256:(dh + 1) * 256],
                                lhsT=bk_sb[0:1, dh * 128: dh * 128 + 128],
                                rhs=ones_sb[0:1, 0:256],
                                start=False, stop=True)
                    kT_sb = workp.tile([128, 512], bf16, tag="kT")
                    nc.vector.tensor_copy(kT_sb[:], kp[:])

                    # V projection -> v natural (blk-major: [b*256 + dout])
                    vp = projp.tile([128, 512], f32, tag="proj")
                    for b in range(2):
                        for kh in range(2):
                            nc.tensor.matmul(
                                out=vp[:, b * 256:(b + 1) * 256],
                                lhsT=kv_sb[:, kh * 256 + b * 128:
                                           kh * 256 + b * 128 + 128],
                                rhs=wv_sb[:, kh * 256:(kh + 1) * 256],
                                start=(kh == 0), stop=(kh == 1 and not use_bv))
                        if use_bv:
                            nc.tensor.matmul(
                                out=vp[:, b * 256:(b + 1) * 256],
                                lhsT=ones_sb[0:1, 0:128],
                                rhs=bv_sb[0:1, :],
                                start=False, stop=True)
                    v_sb = workp.tile([128, 512], bf16, tag="v")
                    nc.vector.tensor_copy(v_sb[:], vp[:])

                    for b in range(2):
                        blk = s * NB + it * 2 + b
                        first = (it == 0 and b == 0)
                        last = (it == NIT - 1 and b == 1)
                        # scores^T [m=128, n] per head; heads 0-3 at cols
                        # h*100, heads 4-7 at 512+(h-4)*100 (bank-safe)
                        spAB = [spp.tile([128, 512], f32, tag="sp",
                                         name=f"sp_{s}_{it}_{b}_{g}")
                                for g in range(2)]
                        for dh in range(2):
                            nc.tensor.matmul(
                                out=spAB[dh][:, 0:400],
                                lhsT=kT_sb[:, dh * 256 + b * 128:
                                           dh * 256 + b * 128 + 128],
                                rhs=qTz[:, (s * H + dh * 4) * NQ:
                                        (s * H + dh * 4 + 4) * NQ],
                                start=True, stop=True)
                        # p = exp(scores + maskbias)  (mask fused via bias)
                        p_sb = pbufp.tile([128, 800], bf16, tag="p")
                        nc.scalar.activation(
                            p_sb[:, 0:400], spAB[0][:, 0:400], Exp,
                            bias=maskb_sb[:, blk:blk + 1], scale=1.0)
                        nc.scalar.activation(
                            p_sb[:, 400:800], spAB[1][:, 0:400], Exp,
                            bias=maskb_sb[:, blk:blk + 1], scale=1.0)
                        # ctx (unnormalized) and l accumulate over blocks
                        for dh in range(2):
                            nc.tensor.matmul(
                                out=cp[dh][:, 0:400],
                                lhsT=v_sb[:, b * 256 + dh * 128:
                                          b * 256 + dh * 128 + 128],
                                rhs=p_sb[:, dh * 400:(dh + 1) * 400],
                                start=first, stop=last)
                            nc.tensor.matmul(
                                out=lp[0:1, dh * 512: dh * 512 + 400],
                                lhsT=ones_sb[:, 0:1],
                                rhs=p_sb[:, dh * 400:(dh + 1) * 400],
                                start=first, stop=last)

                # ---- slot finalize ----
                linv_sb = workp.tile([128, 800], f32, tag="linv")
                for g in range(2):
                    nc.vector.reciprocal(
                        out=linv_sb[0:1, g * 400:(g + 1) * 400],
                        in_=lp[0:1, g * 512: g * 512 + 400])
                # broadcast 1/l down all 128 partitions via K=1 fp32 matmuls
                lb = [spp.tile([128, 512], f32, tag="sp", name=f"lb_{s}_{g}")
                      for g in range(2)]
                for g in range(2):
                    nc.tensor.matmul(
                        out=lb[g][:, 0:400],
                        lhsT=onesf_sb[0:1, 0:128],
                        rhs=linv_sb[0:1, g * 400:(g + 1) * 400],
                        start=True, stop=True)
                lb_sb = workp.tile([128, 800], f32, tag="lbsb")
                for g in range(2):
                    nc.vector.tensor_copy(
                        lb_sb[:, g * 400:(g + 1) * 400], lb[g][:, 0:400])
                # ctxT = ctx_unnorm * (1/l), cast bf16
                ctxT_sb = workp.tile([128, 200], bf16, tag="ctxT")
                for h in range(H):
                    hh = h % 4
                    dh = h // 4
                    nc.vector.tensor_tensor(
                        out=ctxT_sb[hh * 32:hh * 32 + 32, dh * 100:dh * 100 + 100],
                        in0=cp[dh][hh * 32:hh * 32 + 32, hh * 100:hh * 100 + 100],
                        in1=lb_sb[hh * 32:hh * 32 + 32,
                                  dh * 400 + hh * 100:dh * 400 + hh * 100 + 100],
                        op=mybir.AluOpType.mult)
                # out-projection + bo + residual
                op_ps = spp.tile([128, 512], f32, tag="sp")
                for kh in range(2):
                    nc.tensor.matmul(
                        out=op_ps[0:NQ, 0:256],
                        lhsT=ctxT_sb[:, kh * 100: kh * 100 + 100],
                        rhs=wo_sb[:, kh * 256:(kh + 1) * 256],
                        start=(kh == 0), stop=(kh == 1 and not use_bo))
                if use_bo:
                    nc.tensor.matmul(
                        out=op_ps[0:NQ, 0:256],
                        lhsT=ones_sb[0:1, 0:NQ],
                        rhs=bo_sb[0:1, :],
                        start=False, stop=True)
                nc.vector.tensor_tensor(
                    out=out_sb[0:NQ, s * 256:(s + 1) * 256],
                    in0=op_ps[0:NQ, 0:256],
                    in1=qres_sb[0:NQ, s * 256:(s + 1) * 256],
                    op=mybir.AluOpType.add)

            nc.sync.dma_start(
                out=out_d[:].rearrange("(s n) d -> n s d", s=S),
                in_=out_sb[0:NQ, :].rearrange("n (s d) -> n s d", s=S))

    _install_wait_split(nc)
    return nc


def _get_program(Lslot, flags):
    key = (Lslot,) + flags
    if key not in _prog_cache:
        _prog_cache[key] = _build_program(Lslot, *flags)
    return _prog_cache[key]


def kernel(source, query, batch_offsets, Wq, bq, Wk, bk, Wv, bv, Wo, bo):
    from concourse.bass_utils import run_bass_kernel_spmd

    source = np.asarray(source, dtype=np.float32)
    query = np.asarray(query, dtype=np.float32)
    offs = np.asarray(batch_offsets).astype(np.int64)
    Wq = np.asarray(Wq, np.float32); bq = np.asarray(bq, np.float32)
    Wk = np.asarray(Wk, np.float32); bk = np.asarray(bk, np.float32)
    Wv = np.asarray(Wv, np.float32); bv = np.asarray(bv, np.float32)
    Wo = np.asarray(Wo, np.float32); bo = np.asarray(bo, np.float32)
    B = query.shape[0]
    assert B == NCORES * S

    lens = offs[1:] - offs[:-1]
    Lmax = int(lens.max()) if len(lens) else 1
    Lslot = max(256, _ceil_to(max(Lmax, 1), 256))
    NB = Lslot // 128
    T = S * Lslot
    NT = S * NB

    scale = 1.0 / np.sqrt(np.float32(HD))
    flags = (bool(bk.any()), bool(bv.any()), bool(bq.any()), bool(bo.any()))
    nc = _get_program(Lslot, flags)

    wkT = np.ascontiguousarray(Wk.T).astype(BF16)
    wvT = np.ascontiguousarray(Wv.T).astype(BF16)
    wqT = np.ascontiguousarray((Wq * scale).T).astype(BF16)
    woT = np.ascontiguousarray(Wo.T).astype(BF16)
    onesb = np.ones((128, 256), BF16)
    onesf = np.ones((128, 128), np.float32)
    bk_r = bk.reshape(1, D).astype(BF16)
    bv_r = bv.reshape(1, D).astype(BF16)
    bq_r = (bq * scale).reshape(1, D).astype(BF16)
    bo_r = bo.reshape(1, D).astype(BF16)

    in_maps = []
    for c in range(NCORES):
        kvT = np.zeros((D, T), BF16)
        maskb = np.full((128, NT), -1e30, np.float32)
        for s in range(S):
            bidx = c * S + s
            L = int(lens[bidx])
            if L > 0:
                seg = source[offs[bidx]:offs[bidx] + L]
                kvT[:, s * Lslot: s * Lslot + L] = seg.T.astype(BF16)
                nfull = L // 128
                maskb[:, s * NB: s * NB + nfull] = 0.0
                if L % 128:
                    maskb[0:L % 128, s * NB + nfull] = 0.0
        q2 = query[c * S:(c + 1) * S].reshape(S * NQ, D)
        qTin = np.ascontiguousarray(q2.T).astype(BF16)
        qres = np.ascontiguousarray(q2)
        in_maps.append({
            "kvT": kvT, "qTin": qTin, "qres": qres, "maskb": maskb,
            "wkT": wkT, "wvT": wvT, "wqT": wqT, "woT": woT,
            "onesb": onesb, "onesf": onesf,
            "bk": bk_r, "bv": bv_r, "bq": bq_r, "bo": bo_r,
        })

    res = run_bass_kernel_spmd(nc, in_maps, list(range(NCORES)))
    out = np.concatenate(
        [res.results[c]["out"].reshape(S, NQ, D) for c in range(NCORES)],
        axis=0).astype(np.float32)

    # Empty segments: reference attends uniformly over Lmax copies of
    # source[0] -> ctx = v(source[0]); compute exactly on host.
    for bidx in range(B):
        if lens[bidx] == 0:
            v0 = source[0] @ Wv.T + bv
            out[bidx] = (v0 @ Wo.T + bo)[None, :] + query[bidx]

    return out


if __name__ == "__main__":
    # smoke test with tiny random data path is exercised via test.py
    pass


# revision 17
# speedup vs baseline: 542.3036x; 542.3036x over previous
"""Trainium2 Bass kernel for nn_CrossAttentionLayer (ragged cross-attention).

Sharding: data-parallel over the 16 ragged samples -> 2 samples per core
(8 cores). Small attention weights replicated. Host pre-shards source by
batch_offsets (each sample's rows are contiguous), pre-transposes each
core's kv slice to [D, T] and casts to bf16. Device does K/V projections,
scores, softmax (no max-subtraction; additive -1e30 mask fused into the
exp's bias operand), attn@V with fp32 PSUM accumulation, out-projection
and residual. Softmax normalization (division by the per-query sum of
exp) happens at slot finalize on-device.
"""
import sys
import os
import numpy as np

sys.path.insert(0, "/opt/trn_rl_repo")

import ml_dtypes  # noqa: E402

BF16 = ml_dtypes.bfloat16

D = 256
H = 8
HD = 32
NQ = 100
NCORES = 8
S = 2  # sample slots per core

_prog_cache = {}
TRACE_SIM = False


def _ceil_to(x, m):
    return ((x + m - 1) // m) * m


def _patch_tile_drain():
    """walrus CoreV3 CTRL codegen rejects >2 sem-waits on one Drain; the
    Tile kernel-tail drain aggregates one wait per live proc. Split the
    waits across preceding single-wait SP nops instead."""
    from concourse import mybir
    from concourse import tile as tile_mod

    if getattr(tile_mod.TileContext, "_drain_patched", False):
        return

    def _drain_and_barrier(self, tick_clock, wait_clock):
        nc = self.nc
        carrier = nc.sync.nop(nofuse=True)
        wait_clock.add_sem_waits(
            carrier.ins, tile_mod.ScopedClock({None: tick_clock.global_clock}))
        si = carrier.ins.sync_info
        waits = list(si.on_wait) if si and si.on_wait else []
        MAXW = 1
        if len(waits) > MAXW:
            si.on_wait = waits[:MAXW]
            for i in range(MAXW, len(waits), MAXW):
                nop = nc.sync.nop(nofuse=True)
                nop.ins.sync_info = mybir.SyncInfo(
                    on_wait=waits[i:i + MAXW], on_update=[])
        nc.sync.drain()
        nc.all_engine_barrier()
        popped = nc._tile_sem_poison_stack.pop()
        assert popped is self._sem_poison
        nc.clear_and_free_semaphores(list(self.sems.allocated().values()))
        nc.all_engine_barrier()

    tile_mod.TileContext._drain_and_barrier = _drain_and_barrier
    tile_mod.TileContext._drain_patched = True


def _split_bir_waits(m, maxw=1):
    """walrus CoreV2/V3 codegen rejects instructions carrying more than one
    sync-wait command. Hoist extra waits onto same-engine NoOps inserted
    immediately before the instruction (engine execution is in-order, so
    the happens-before is preserved)."""
    uid = [0]
    for fn in m.get("functions", []):
        for bb in fn.get("blocks", []):
            out = []
            for ins in bb.get("instructions", []):
                si = ins.get("sync_info")
                waits = (si or {}).get("on_wait") or []
                if len(waits) > maxw:
                    for i in range(0, len(waits) - maxw, maxw):
                        uid[0] += 1
                        out.append({
                            "debug": ins.get("debug", 0),
                            "engine": ins["engine"],
                            "ins": [],
                            "name": f"{ins['name']}-w{uid[0]}",
                            "opcode": "NoOp",
                            "outs": [],
                            "sync_info": {
                                "on_update": [],
                                "on_wait": waits[i:i + maxw],
                            },
                        })
                    si["on_wait"] = waits[len(waits) - maxw:]
                out.append(ins)
            bb["instructions"] = out
    return m


def _install_wait_split(nc):
    import orjson
    orig = nc.to_json_bytes

    def patched():
        return orjson.dumps(_split_bir_waits(orjson.loads(orig())))

    nc.to_json_bytes = patched


def _build_program(Lslot, use_bk, use_bv, use_bq, use_bo):
    """Build the SPMD Bass program for one core handling S=2 slots of
    Lslot (multiple of 256) padded kv tokens each."""
    from concourse import bass, mybir
    from concourse.tile import TileContext

    _patch_tile_drain()

    f32 = mybir.dt.float32
    bf16 = mybir.dt.bfloat16
    Exp = mybir.ActivationFunctionType.Exp

    NB = Lslot // 128          # 128-token blocks per slot
    NIT = Lslot // 256         # 256-token iterations per slot
    T = S * Lslot
    NT = S * NB

    nc = bass.Bass()

    kvT_d = nc.declare_dram_parameter("kvT", [D, T], bf16, isOutput=False)
    qTin_d = nc.declare_dram_parameter("qTin", [D, S * NQ], bf16, isOutput=False)
    qres_d = nc.declare_dram_parameter("qres", [S * NQ, D], f32, isOutput=False)
    maskb_d = nc.declare_dram_parameter("maskb", [128, NT], f32, isOutput=False)
    wkT_d = nc.declare_dram_parameter("wkT", [D, D], bf16, isOutput=False)
    wvT_d = nc.declare_dram_parameter("wvT", [D, D], bf16, isOutput=False)
    wqT_d = nc.declare_dram_parameter("wqT", [D, D], bf16, isOutput=False)
    woT_d = nc.declare_dram_parameter("woT", [D, D], bf16, isOutput=False)
    ones_d = nc.declare_dram_parameter("onesb", [128, 256], bf16, isOutput=False)
    onesf_d = nc.declare_dram_parameter("onesf", [128, 128], f32, isOutput=False)
    bk_d = nc.declare_dram_parameter("bk", [1, D], bf16, isOutput=False)
    bv_d = nc.declare_dram_parameter("bv", [1, D], bf16, isOutput=False)
    bq_d = nc.declare_dram_parameter("bq", [1, D], bf16, isOutput=False)
    bo_d = nc.declare_dram_parameter("bo", [1, D], bf16, isOutput=False)
    out_d = nc.declare_dram_parameter("out", [S * NQ, D], f32, isOutput=True)

    with TileContext(nc, trace_sim=TRACE_SIM) as tc:
        with tc.tile_pool(name="const", bufs=1) as cpool, \
             tc.tile_pool(name="proj", bufs=2, space="PSUM") as projp, \
             tc.tile_pool(name="sp", bufs=2, space="PSUM") as spp, \
             tc.tile_pool(name="cp", bufs=2, space="PSUM") as cpp, \
             tc.tile_pool(name="lp", bufs=1, space="PSUM") as lpp, \
             tc.tile_pool(name="load", bufs=4) as loadp, \
             tc.tile_pool(name="work", bufs=3) as workp, \
             tc.tile_pool(name="pbuf", bufs=4) as pbufp:

            # ---- constants / small tensors ----
            wk_sb = cpool.tile([128, 512], bf16)
            wv_sb = cpool.tile([128, 512], bf16)
            wq_sb = cpool.tile([128, 512], bf16)
            wo_sb = cpool.tile([128, 512], bf16)
            qTin_sb = cpool.tile([128, 2 * S * NQ], bf16)
            qres_sb = cpool.tile([128, S * D], f32)
            maskb_sb = cpool.tile([128, NT], f32)
            ones_sb = cpool.tile([128, 256], bf16)
            onesf_sb = cpool.tile([128, 128], f32)
            bk_sb = cpool.tile([1, D], bf16)
            bv_sb = cpool.tile([1, D], bf16)
            bq_sb = cpool.tile([1, D], bf16)
            bo_sb = cpool.tile([1, D], bf16)
            qTz = cpool.tile([128, S * H * NQ], bf16)
            out_sb = cpool.tile([128, S * D], f32)

            for w_sb, w_d in ((wk_sb, wkT_d), (wv_sb, wvT_d),
                              (wq_sb, wqT_d), (wo_sb, woT_d)):
                nc.scalar.dma_start(
                    out=w_sb[:].rearrange("p (kh d) -> p kh d", kh=2),
                    in_=w_d[:].rearrange("(kh p) d -> p kh d", kh=2))
            nc.scalar.dma_start(
                out=qTin_sb[:].rearrange("p (kh n) -> p kh n", kh=2),
                in_=qTin_d[:].rearrange("(kh p) n -> p kh n", kh=2))
            nc.scalar.dma_start(
                out=qres_sb[0:NQ, :].rearrange("n (s d) -> n s d", s=S),
                in_=qres_d[:].rearrange("(s n) d -> n s d", s=S))
            nc.scalar.dma_start(out=maskb_sb[:], in_=maskb_d[:])
            nc.scalar.dma_start(out=ones_sb[:], in_=ones_d[:])
            nc.scalar.dma_start(out=onesf_sb[:], in_=onesf_d[:])
            for b_sb, b_d in ((bk_sb, bk_d), (bv_sb, bv_d),
                              (bq_sb, bq_d), (bo_sb, bo_d)):
                nc.scalar.dma_start(out=b_sb[:], in_=b_d[:])

            # ---- q projection (once): qT[dout, n] over dh halves ----
            qp = spp.tile([128, 512], f32, tag="sp", name="qp")
            NQS = S * NQ
            for dh in range(2):
                for kh in range(2):
                    nc.tensor.matmul(
                        out=qp[:, dh * NQS:(dh + 1) * NQS],
                        lhsT=wq_sb[:, kh * 256 + dh * 128: kh * 256 + dh * 128 + 128],
                        rhs=qTin_sb[:, kh * NQS:(kh + 1) * NQS],
                        start=(kh == 0), stop=(kh == 1 and not use_bq))
                if use_bq:
                    nc.tensor.matmul(
                        out=qp[:, dh * NQS:(dh + 1) * NQS],
                        lhsT=bq_sb[0:1, dh * 128: dh * 128 + 128],
                        rhs=ones_sb[0:1, 0:NQS],
                        start=False, stop=True)
            nc.gpsimd.memset(qTz[:], 0.0)
            for s in range(S):
                for h in range(H):
                    rp = (h % 4) * 32
                    nc.vector.tensor_copy(
                        qTz[rp:rp + 32, (s * H + h) * NQ:(s * H + h) * NQ + NQ],
                        qp[rp:rp + 32, (h // 4) * NQS + s * NQ:
                           (h // 4) * NQS + s * NQ + NQ])

            # ---- main loop over slots and 256-token iterations ----
            for s in range(S):
                cp = [cpp.tile([128, 512], f32, tag="cp", name=f"cp_s{s}_t{t}")
                      for t in range(2)]
                lp = lpp.tile([128, 1024], f32, tag="lp", name=f"lp_s{s}")
                for it in range(NIT):
                    c0 = s * Lslot + it * 256
                    kv_sb = loadp.tile([128, 512], bf16, tag="kv")
                    nc.sync.dma_start(
                        out=kv_sb[:].rearrange("p (kh m) -> p kh m", kh=2),
                        in_=kvT_d[:, c0:c0 + 256].rearrange(
                            "(kh p) m -> p kh m", kh=2))

                    # K projection -> kT (dh-major: [dh*256 + m(2blk)])
                    kp = projp.tile([128, 512], f32, tag="proj")
                    for dh in range(2):
                        for kh in range(2):
                            nc.tensor.matmul(
                                out=kp[:, dh * 256:(dh + 1) * 256],
                                lhsT=wk_sb[:, kh * 256 + dh * 128:
                                           kh * 256 + dh * 128 + 128],
                                rhs=kv_sb[:, kh * 256:(kh + 1) * 256],
                                start=(kh == 0), stop=(kh == 1 and not use_bk))
                        if use_bk:
                            nc.tensor.matmul(
                                out=kp[:, dh * 256:(dh + 1) * 256],
                                lhsT=bk_sb[0:1, dh * 128: dh * 128 + 128],
                                rhs=ones_sb[0:1, 0:256],
                                start=False, stop=True)
                    kT_sb = workp.tile([128, 512], bf16, tag="kT")
                    nc.vector.tensor_copy(kT_sb[:], kp[:])

                    # V projection -> v natural (blk-major: [b*256 + dout])
                    vp = projp.tile([128, 512], f32, tag="proj")
                    for b in range(2):
                        for kh in range(2):
                            nc.tensor.matmul(
                                out=vp[:, b * 256:(b + 1) * 256],
                                lhsT=kv_sb[:, kh * 256 + b * 128:
                                           kh * 256 + b * 128 + 128],
                                rhs=wv_sb[:, kh * 256:(kh + 1) * 256],
                                start=(kh == 0), stop=(kh == 1 and not use_bv))
                        if use_bv:
                            nc.tensor.matmul(
                                out=vp[:, b * 256:(b + 1) * 256],
                                lhsT=ones_sb[0:1, 0:128],
                                rhs=bv_sb[0:1, :],
                                start=False, stop=True)
                    v_sb = workp.tile([128, 512], bf16, tag="v")
                    nc.vector.tensor_copy(v_sb[:], vp[:])

                    for b in range(2):
                        blk = s * NB + it * 2 + b
                        first = (it == 0 and b == 0)
                        last = (it == NIT - 1 and b == 1)
                        # scores^T [m=128, n] per head; heads 0-3 at cols
                        # h*100, heads 4-7 at 512+(h-4)*100 (bank-safe)
                        spAB = [spp.tile([128, 512], f32, tag="sp",
                                         name=f"sp_{s}_{it}_{b}_{g}")
                                for g in range(2)]
                        for dh in range(2):
                            nc.tensor.matmul(
                                out=spAB[dh][:, 0:400],
                                lhsT=kT_sb[:, dh * 256 + b * 128:
                                           dh * 256 + b * 128 + 128],
                                rhs=qTz[:, (s * H + dh * 4) * NQ:
                                        (s * H + dh * 4 + 4) * NQ],
                                start=True, stop=True)
                        # p = exp(scores + maskbias)  (mask fused via bias)
                        p_sb = pbufp.tile([128, 800], bf16, tag="p")
                        nc.scalar.activation(
                            p_sb[:, 0:400], spAB[0][:, 0:400], Exp,
                            bias=maskb_sb[:, blk:blk + 1], scale=1.0)
                        nc.scalar.activation(
                            p_sb[:, 400:800], spAB[1][:, 0:400], Exp,
                            bias=maskb_sb[:, blk:blk + 1], scale=1.0)
                        # ctx (unnormalized) and l accumulate over blocks
                        for dh in range(2):
                            nc.tensor.matmul(
                                out=cp[dh][:, 0:400],
                                lhsT=v_sb[:, b * 256 + dh * 128:
                                          b * 256 + dh * 128 + 128],
                                rhs=p_sb[:, dh * 400:(dh + 1) * 400],
                                start=first, stop=last)
                            nc.tensor.matmul(
                                out=lp[0:1, dh * 512: dh * 512 + 400],
                                lhsT=ones_sb[:, 0:1],
                                rhs=p_sb[:, dh * 400:(dh + 1) * 400],
                                start=first, stop=last)

                # ---- slot finalize ----
                linv_sb = workp.tile([128, 800], f32, tag="linv")
                for g in range(2):
                    nc.vector.reciprocal(
                        out=linv_sb[0:1, g * 400:(g + 1) * 400],
                        in_=lp[0:1, g * 512: g * 512 + 400])
                # broadcast 1/l down all 128 partitions via K=1 fp32 matmuls
                lb = [lpp.tile([128, 512], f32, tag="lp", name=f"lb_{s}_{g}")
                      for g in range(2)]
                for g in range(2):
                    nc.tensor.matmul(
                        out=lb[g][:, 0:400],
                        lhsT=onesf_sb[0:1, 0:128],
                        rhs=linv_sb[0:1, g * 400:(g + 1) * 400],
                        start=True, stop=True)
                lb_sb = workp.tile([128, 800], f32, tag="lbsb")
                for g in range(2):
                    nc.vector.tensor_copy(
                        lb_sb[:, g * 400:(g + 1) * 400], lb[g][:, 0:400])
                # ctxT = ctx_unnorm * (1/l), cast bf16
                ctxT_sb = workp.tile([128, 200], bf16, tag="ctxT")
                for h in range(H):
                    hh = h % 4
                    dh = h // 4
                    nc.vector.tensor_tensor(
                        out=ctxT_sb[hh * 32:hh * 32 + 32, dh * 100:dh * 100 + 100],
                        in0=cp[dh][hh * 32:hh * 32 + 32, hh * 100:hh * 100 + 100],
                        in1=lb_sb[hh * 32:hh * 32 + 32,
                                  dh * 400 + hh * 100:dh * 400 + hh * 100 + 100],
                        op=mybir.AluOpType.mult)
                # out-projection + bo + residual
                op_ps = lpp.tile([128, 512], f32, tag="lp", name=f"op_{s}")
                for kh in range(2):
                    nc.tensor.matmul(
                        out=op_ps[0:NQ, 0:256],
                        lhsT=ctxT_sb[:, kh * 100: kh * 100 + 100],
                        rhs=wo_sb[:, kh * 256:(kh + 1) * 256],
                        start=(kh == 0), stop=(kh == 1 and not use_bo))
                if use_bo:
                    nc.tensor.matmul(
                        out=op_ps[0:NQ, 0:256],
                        lhsT=ones_sb[0:1, 0:NQ],
                        rhs=bo_sb[0:1, :],
                        start=False, stop=True)
                nc.vector.tensor_tensor(
                    out=out_sb[0:NQ, s * 256:(s + 1) * 256],
                    in0=op_ps[0:NQ, 0:256],
                    in1=qres_sb[0:NQ, s * 256:(s + 1) * 256],
                    op=mybir.AluOpType.add)

            nc.sync.dma_start(
                out=out_d[:].rearrange("(s n) d -> n s d", s=S),
                in_=out_sb[0:NQ, :].rearrange("n (s d) -> n s d", s=S))

    _install_wait_split(nc)
    return nc


def _get_program(Lslot, flags):
    key = (Lslot,) + flags
    if key not in _prog_cache:
        _prog_cache[key] = _build_program(Lslot, *flags)
    return _prog_cache[key]


def kernel(source, query, batch_offsets, Wq, bq, Wk, bk, Wv, bv, Wo, bo):
    from concourse.bass_utils import run_bass_kernel_spmd

    source = np.asarray(source, dtype=np.float32)
    query = np.asarray(query, dtype=np.float32)
    offs = np.asarray(batch_offsets).astype(np.int64)
    Wq = np.asarray(Wq, np.float32); bq = np.asarray(bq, np.float32)
    Wk = np.asarray(Wk, np.float32); bk = np.asarray(bk, np.float32)
    Wv = np.asarray(Wv, np.float32); bv = np.asarray(bv, np.float32)
    Wo = np.asarray(Wo, np.float32); bo = np.asarray(bo, np.float32)
    B = query.shape[0]
    assert B == NCORES * S

    lens = offs[1:] - offs[:-1]
    Lmax = int(lens.max()) if len(lens) else 1
    Lslot = max(256, _ceil_to(max(Lmax, 1), 256))
    NB = Lslot // 128
    T = S * Lslot
    NT = S * NB

    scale = 1.0 / np.sqrt(np.float32(HD))
    flags = (bool(bk.any()), bool(bv.any()), bool(bq.any()), bool(bo.any()))
    nc = _get_program(Lslot, flags)

    wkT = np.ascontiguousarray(Wk.T).astype(BF16)
    wvT = np.ascontiguousarray(Wv.T).astype(BF16)
    wqT = np.ascontiguousarray((Wq * scale).T).astype(BF16)
    woT = np.ascontiguousarray(Wo.T).astype(BF16)
    onesb = np.ones((128, 256), BF16)
    onesf = np.ones((128, 128), np.float32)
    bk_r = bk.reshape(1, D).astype(BF16)
    bv_r = bv.reshape(1, D).astype(BF16)
    bq_r = (bq * scale).reshape(1, D).astype(BF16)
    bo_r = bo.reshape(1, D).astype(BF16)

    in_maps = []
    for c in range(NCORES):
        kvT = np.zeros((D, T), BF16)
        maskb = np.full((128, NT), -1e30, np.float32)
        for s in range(S):
            bidx = c * S + s
            L = int(lens[bidx])
            if L > 0:
                seg = source[offs[bidx]:offs[bidx] + L]
                kvT[:, s * Lslot: s * Lslot + L] = seg.T.astype(BF16)
                nfull = L // 128
                maskb[:, s * NB: s * NB + nfull] = 0.0
                if L % 128:
                    maskb[0:L % 128, s * NB + nfull] = 0.0
        q2 = query[c * S:(c + 1) * S].reshape(S * NQ, D)
        qTin = np.ascontiguousarray(q2.T).astype(BF16)
        qres = np.ascontiguousarray(q2)
        in_maps.append({
            "kvT": kvT, "qTin": qTin, "qres": qres, "maskb": maskb,
            "wkT": wkT, "wvT": wvT, "wqT": wqT, "woT": woT,
            "onesb": onesb, "onesf": onesf,
            "bk": bk_r, "bv": bv_r, "bq": bq_r, "bo": bo_r,
        })

    res = run_bass_kernel_spmd(nc, in_maps, list(range(NCORES)))
    out = np.concatenate(
        [res.results[c]["out"].reshape(S, NQ, D) for c in range(NCORES)],
        axis=0).astype(np.float32)

    # Empty segments: reference attends uniformly over Lmax copies of
    # source[0] -> ctx = v(source[0]); compute exactly on host.
    for bidx in range(B):
        if lens[bidx] == 0:
            v0 = source[0] @ Wv.T + bv
            out[bidx] = (v0 @ Wo.T + bo)[None, :] + query[bidx]

    return out


if __name__ == "__main__":
    # smoke test with tiny random data path is exercised via test.py
    pass
